# revision 1
# baseline (speedup 1.0000x reference)
"""Trainium2 Bass kernel for ConvaiCausalLMAttention (GQA causal attention).

Problem: B=1, S=4096, H=2048, 32 query heads, 8 KV heads, head_dim 64.
  q = x @ Wq.T ; k = x @ Wk.T ; v = x @ Wv.T  (nn.Linear convention)
  attn = softmax(q k^T / 8 + causal_mask) v ;  out = attn @ Wo.T

Sharding: tensor-parallel across heads over 8 cores. Core c owns query
heads [4c, 4c+4) (256 q-dims) and exactly KV head c (GQA group == core).
Each core computes its 4 heads' attention plus a partial output
projection against its 256 columns of Wo; the host sums the 8 partials.

Device-side design (matmuls in bf16, fp32 PSUM accumulation):
  - X is fed transposed (X^T [2048, 4096]) so projections need no
    on-device transpose: Q^T = (Wq/8) X^T, K^T = Wk X^T (duplicated into
    both partition halves so head-pair score matmuls pack into both
    halves of the PE array), V^T then PE-transposed into [keys, dim].
  - Scores are computed transposed (S^T[k, q] = K Q^T) so softmax'd
    probabilities feed the PV matmul directly as the moving operand.
  - Softmax skips max-subtraction (scores are O(10), exp is safe in
    fp32) and gets the denominator for free from an all-ones column
    embedded in the V tile; causal masking multiplies exp by a 0/1 mask
    on the four diagonal key-chunks only; key chunks above the diagonal
    are skipped entirely (2x flop saving).
  - Engines are in-order queues, so projection/Wo matmul chains are
    interleaved into the ACT-bound attention stream to keep the PE busy
    while exp() runs.
"""

import numpy as np
import ml_dtypes

import concourse.bass as bass
import concourse.mybir as mybir
from concourse import bacc
from concourse.tile import TileContext
from concourse.bass_utils import run_bass_kernel_spmd

F32 = mybir.dt.float32
F32R = mybir.dt.float32r
BF16 = mybir.dt.bfloat16

H = 2048
S = 4096
NH = 32
NKV = 8
HD = 64
NCORES = 8
QD = 256          # query dims per core (4 heads x 64)
NSC = 8           # 512-wide sequence chunks
SC = 512          # seq chunk width
KC = 128          # key chunk width
HB = H // 128     # 16 contraction blocks


def _build_nc(reps: int = 1):
    nc = bacc.Bacc("TRN2", target_bir_lowering=False, debug=False,
                   num_devices=NCORES)

    xt = nc.declare_dram_parameter("xt", [H, S], BF16, isOutput=False)
    wq = nc.declare_dram_parameter("wq", [H, QD], BF16, isOutput=False)
    wk = nc.declare_dram_parameter("wk", [H, 128], BF16, isOutput=False)
    wv = nc.declare_dram_parameter("wv", [H, HD], BF16, isOutput=False)
    wo = nc.declare_dram_parameter("wo", [QD, H], BF16, isOutput=False)
    masks = nc.declare_dram_parameter("masks", [128, 4, SC], BF16, isOutput=False)
    ones = nc.declare_dram_parameter("ones", [128, 128], F32R, isOutput=False)
    eye = nc.declare_dram_parameter("eye", [64, 64], F32, isOutput=False)
    out = nc.declare_dram_parameter("out", [S, H], F32, isOutput=True)

    xt_r = xt.rearrange("(hb p) s -> p hb s", p=128)
    wq_r = wq.rearrange("(hb p) m -> p hb m", p=128)
    wk_r = wk.rearrange("(hb p) m -> p hb m", p=128)
    wv_r = wv.rearrange("(hb p) m -> p hb m", p=128)
    wo_r = wo.rearrange("(b p) n -> p b n", p=128)

    EXP = mybir.ActivationFunctionType.Exp

    with TileContext(nc) as tc, \
         tc.tile_pool(name="persist", bufs=1) as persist, \
         tc.tile_pool(name="xtp", bufs=3) as xtp, \
         tc.tile_pool(name="qtp", bufs=2) as qtp, \
         tc.tile_pool(name="vtp", bufs=2) as vtp, \
         tc.tile_pool(name="expp", bufs=4) as expp, \
         tc.tile_pool(name="attp", bufs=2) as attp, \
         tc.tile_pool(name="recp", bufs=3) as recp, \
         tc.tile_pool(name="outp", bufs=4) as outp, \
         tc.tile_pool(name="ps_proj", bufs=2, space="PSUM") as ps_proj, \
         tc.tile_pool(name="ps_sc", bufs=2, space="PSUM") as ps_sc, \
         tc.tile_pool(name="ps_pv", bufs=2, space="PSUM") as ps_pv:

        # ---- persistent tiles ----
        wq_sb = persist.tile([128, HB, QD], BF16)
        wk_sb = persist.tile([128, HB, 128], BF16)
        wv_sb = persist.tile([128, HB, HD], BF16)
        wo_sb = persist.tile([128, 2, H], BF16)
        mask_sb = persist.tile([128, 4, SC], BF16)
        ones_sb = persist.tile([128, 128], F32R)
        eye_sb = persist.tile([64, 64], F32)
        ktdup = persist.tile([128, S], BF16)          # [dup(64)|dup(64), keys]
        # V buffer: per key chunk 160 cols: [V(64) | ones | junk(31) | V(64)]
        vbuf = persist.tile([128, S // KC, 160], BF16)

        nc.sync.dma_start(wq_sb[:], wq_r)
        nc.sync.dma_start(wk_sb[:], wk_r)
        nc.sync.dma_start(wv_sb[:], wv_r)
        nc.sync.dma_start(wo_sb[:], wo_r)
        nc.sync.dma_start(mask_sb[:], masks[:])
        nc.sync.dma_start(ones_sb[:], ones[:])
        nc.sync.dma_start(eye_sb[:], eye[:])
        # ones column of vbuf (denominator trick)
        for kc in range(S // KC):
            nc.vector.tensor_copy(vbuf[:, kc, 64:65], ones_sb[:, 0:1])

        # ---------- emission helpers ----------
        xt_tiles = {}

        def emit_xt_dma(s):
            q0 = s * SC
            lo = xtp.tile([128, HB // 2, SC], BF16, tag="xt", name="xt_lo")
            hi = xtp.tile([128, HB // 2, SC], BF16, tag="xt", name="xt_hi")
            nc.sync.dma_start(lo[:], xt_r[:, 0:HB // 2, q0:q0 + SC])
            nc.sync.dma_start(hi[:], xt_r[:, HB // 2:HB, q0:q0 + SC])
            xt_tiles[s] = (lo, hi)

        def xchunk(s, hb):
            lo, hi = xt_tiles[s]
            return lo[:, hb, :] if hb < HB // 2 else hi[:, hb - HB // 2, :]

        qt_tiles = {}
        attn_tiles = {}

        def proj_quanta(s):
            """Generator of emission quanta for chunk s's projections."""
            q0 = s * SC

            # Q^T
            qt_sb = qtp.tile([128, 2, SC], BF16, tag="qt")
            qt_tiles[s] = qt_sb
            for m in range(2):
                ps3 = ps_proj.tile([128, SC], F32, tag="proj", name="ps_qt")
                for h0 in range(0, HB, 4):
                    for hb in range(h0, h0 + 4):
                        nc.tensor.matmul(ps3[:],
                                         wq_sb[:, hb, m * 128:(m + 1) * 128],
                                         xchunk(s, hb),
                                         start=(hb == 0), stop=(hb == HB - 1))
                    yield
                nc.vector.tensor_copy(qt_sb[:, m, :], ps3[:])

            # K^T duplicated -> ktdup[:, q0:q0+SC]
            ps = ps_proj.tile([128, SC], F32, tag="proj", name="ps_kt")
            for h0 in range(0, HB, 4):
                for hb in range(h0, h0 + 4):
                    nc.tensor.matmul(ps[:], wk_sb[:, hb, :], xchunk(s, hb),
                                     start=(hb == 0), stop=(hb == HB - 1))
                yield
            nc.vector.tensor_copy(ktdup[:, q0:q0 + SC], ps[:])

            # V^T staging
            ps2 = ps_proj.tile([128, SC], F32, tag="proj", name="ps_vt")
            for h0 in range(0, HB, 4):
                for hb in range(h0, h0 + 4):
                    nc.tensor.matmul(ps2[0:64, :], wv_sb[:, hb, :], xchunk(s, hb),
                                     start=(hb == 0), stop=(hb == HB - 1))
                yield
            vt_sb = vtp.tile([64, SC], F32, tag="vt")
            nc.vector.tensor_copy(vt_sb[:], ps2[0:64, :])

            # V transpose into vbuf
            for t in range(SC // KC):
                kc = (q0 // KC) + t
                pst = ps_proj.tile([128, 64], F32, tag="proj", name="ps_tr")
                nc.tensor.transpose(pst[:], vt_sb[:, t * KC:(t + 1) * KC],
                                    eye_sb[:])
                nc.vector.tensor_copy(vbuf[:, kc, 0:64], pst[:])
                nc.vector.tensor_copy(vbuf[:, kc, 96:160], pst[:])
                if t % 2 == 1:
                    yield

        def wo_quanta(s):
            """Generator of emission quanta for chunk s's output projection."""
            attn_sb = attn_tiles[s]
            q0 = s * SC
            for t in range(SC // 128):
                r0 = q0 + t * 128
                for nck in range(H // SC):
                    ps4 = ps_proj.tile([128, SC], F32, tag="proj", name="ps_wo")
                    for b in range(2):
                        nc.tensor.matmul(
                            ps4[:], attn_sb[:, b, t * 128:(t + 1) * 128],
                            wo_sb[:, b, nck * SC:(nck + 1) * SC],
                            start=(b == 0), stop=(b == 1))
                    o_sb = outp.tile([128, SC], F32, tag="out")
                    nc.vector.tensor_copy(o_sb[:], ps4[:])
                    nc.sync.dma_start(
                        out[r0:r0 + 128, nck * SC:(nck + 1) * SC], o_sb[:])
                    yield

        def attention(s, fillers, n_fill):
            """Attention for query chunk s; runs filler quanta between
            score groups so the PE queue stays busy during exp()."""

            def fill(k):
                while k > 0 and fillers:
                    try:
                        next(fillers[0])
                        k -= 1
                    except StopIteration:
                        fillers.pop(0)

            attn_sb = attp.tile([128, 2, SC], BF16, tag="attn")
            attn_tiles[s] = attn_sb
            ngroups = 2 * s + 2
            # spread the filler quanta evenly over the 2*ngroups group slots
            rate = n_fill / (2.0 * ngroups)
            facc = 0.0
            for m in range(2):
                pv_e = ps_pv.tile([128, SC], F32, tag="pv", name="pv_e")
                pv_o = ps_pv.tile([128, SC], F32, tag="pv", name="pv_o")

                exps = {}

                def emit_scores(g):
                    sc_e = ps_sc.tile([128, 2, SC], F32, tag="sc", name="sc_e")
                    sc_o = ps_sc.tile([128, 2, SC], F32, tag="sc", name="sc_o")
                    for j in range(2):
                        kc = 2 * g + j
                        ksl = slice(kc * KC, (kc + 1) * KC)
                        nc.tensor.matmul(sc_e[:, j, :], ktdup[0:64, ksl],
                                         qt_tiles[s][0:64, m, :],
                                         start=True, stop=True)
                        nc.tensor.matmul(sc_o[:, j, :], ktdup[64:128, ksl],
                                         qt_tiles[s][64:128, m, :],
                                         start=True, stop=True)
                    exp_e = expp.tile([128, 2, SC], BF16, tag="exp",
                                      name="exp_e")
                    exp_o = expp.tile([128, 2, SC], BF16, tag="exp",
                                      name="exp_o")
                    nc.scalar.activation(exp_e[:], sc_e[:], EXP)
                    nc.scalar.activation(exp_o[:], sc_o[:], EXP)
                    if g >= ngroups - 2:
                        joff = (g - (ngroups - 2)) * 2
                        nc.vector.tensor_mul(exp_e[:], exp_e[:],
                                             mask_sb[:, joff:joff + 2, :])
                        nc.vector.tensor_mul(exp_o[:], exp_o[:],
                                             mask_sb[:, joff:joff + 2, :])
                    exps[g] = (exp_e, exp_o)

                # software pipeline: scores one group ahead of PV so the PE
                # never sits in an exp() wait.
                emit_scores(0)
                for g in range(ngroups):
                    if g + 1 < ngroups:
                        emit_scores(g + 1)
                    facc += rate
                    nf = int(facc)
                    facc -= nf
                    fill(nf)
                    exp_e, exp_o = exps.pop(g)
                    last = (g == ngroups - 1)
                    for j in range(2):
                        kc = 2 * g + j
                        nc.tensor.matmul(pv_e[:], vbuf[:, kc, 0:128],
                                         exp_e[:, j, :],
                                         start=(g == 0 and j == 0),
                                         stop=(last and j == 1))
                        nc.tensor.matmul(pv_o[:], vbuf[:, kc, 32:160],
                                         exp_o[:, j, :],
                                         start=(g == 0 and j == 0),
                                         stop=(last and j == 1))
                # evacuate PV psum right away so the accumulator banks free
                # for the next head pair; normalize runs from SBUF after.
                pvs = recp.tile([128, 2, SC], F32, tag="pvs")
                nc.vector.tensor_copy(pvs[:, 0, :], pv_e[:])
                nc.vector.tensor_copy(pvs[:, 1, :], pv_o[:])
                # normalize: even head denom at row 64 (j=0), odd at row 32 (j=1)
                rec = recp.tile([128, SC], F32R, tag="rec")
                with nc.allow_low_precision(reason="fp32r recip -> matmul"):
                    nc.vector.reciprocal(rec[64:65, :], pvs[64:65, 0, :])
                    nc.vector.reciprocal(rec[32:33, :], pvs[32:33, 1, :])
                bc_e = ps_proj.tile([128, SC], F32, tag="proj", name="bc_e")
                bc_o = ps_proj.tile([128, SC], F32, tag="proj", name="bc_o")
                nc.tensor.matmul(bc_e[:], ones_sb[64:65, :], rec[64:65, :],
                                 start=True, stop=True)
                nc.tensor.matmul(bc_o[:], ones_sb[32:33, :], rec[32:33, :],
                                 start=True, stop=True)
                bcs = recp.tile([128, SC], F32, tag="bcs")
                nc.vector.tensor_copy(bcs[0:64, :], bc_e[0:64, :])
                nc.vector.tensor_copy(bcs[64:128, :], bc_o[64:128, :])
                nc.vector.tensor_mul(attn_sb[0:64, m, :], pvs[0:64, 0, :],
                                     bcs[0:64, :])
                nc.vector.tensor_mul(attn_sb[64:128, m, :], pvs[64:128, 1, :],
                                     bcs[64:128, :])

        def body():
            qt_tiles.clear()
            attn_tiles.clear()
            xt_tiles.clear()
            emit_xt_dma(0)
            for q in proj_quanta(0):
                pass
            for s in range(NSC):
                fillers = []
                n_fill = 0
                if s + 1 < NSC:
                    emit_xt_dma(s + 1)
                    fillers.append(proj_quanta(s + 1))
                    n_fill += 18
                if s - 1 >= 0:
                    fillers.append(wo_quanta(s - 1))
                    n_fill += 16
                attention(s, fillers, n_fill)
                # drain leftover fillers
                for f in list(fillers):
                    for _ in f:
                        pass
            for _ in wo_quanta(NSC - 1):
                pass

        if reps > 1:
            with tc.For_i(0, reps, 1):
                body()
        else:
            body()

    nc.compile()
    return nc


_NC_CACHE = None


def _get_nc():
    global _NC_CACHE
    if _NC_CACHE is None:
        _NC_CACHE = _build_nc()
    return _NC_CACHE


def _make_in_maps(hidden_states, Wq, Wk, Wv, Wo):
    bf = ml_dtypes.bfloat16
    x = np.asarray(hidden_states, dtype=np.float32).reshape(S, H)
    xt = np.ascontiguousarray(x.T).astype(bf)

    p = np.arange(128)[:, None]
    f = np.arange(SC)[None, :]
    masks = np.stack([(f >= p + 128 * j) for j in range(4)], axis=1)
    masks = np.ascontiguousarray(masks).astype(bf)
    onesv = np.ones((128, 128), np.float32)
    eyev = np.eye(64, dtype=np.float32)

    in_maps = []
    for c in range(NCORES):
        wq_c = np.ascontiguousarray((Wq[QD * c:QD * (c + 1), :] / 8.0).T).astype(bf)
        wk_c = Wk[HD * c:HD * (c + 1), :]
        wk_c = np.ascontiguousarray(np.concatenate([wk_c, wk_c], axis=0).T).astype(bf)
        wv_c = np.ascontiguousarray(Wv[HD * c:HD * (c + 1), :].T).astype(bf)
        wo_c = np.ascontiguousarray(Wo[:, QD * c:QD * (c + 1)].T).astype(bf)
        in_maps.append({
            "xt": xt, "wq": wq_c, "wk": wk_c, "wv": wv_c, "wo": wo_c,
            "masks": masks, "ones": onesv, "eye": eyev,
        })
    return in_maps


def kernel(hidden_states, attention_mask, Wq, Wk, Wv, Wo, _trace=False):
    nc = _get_nc()
    in_maps = _make_in_maps(hidden_states, Wq, Wk, Wv, Wo)
    res = run_bass_kernel_spmd(nc, in_maps, core_ids=list(range(NCORES)),
                               trace=_trace)
    kernel.last_results = res
    total = res.results[0]["out"].astype(np.float32)
    for c in range(1, NCORES):
        total += res.results[c]["out"]
    return total.reshape(1, S, H)



# revision 18
# speedup vs baseline: 50432.0608x; 50432.0608x over previous
"""Trainium2 Bass kernel for ConvaiCausalLMAttention (GQA causal attention).

Problem: B=1, S=4096, H=2048, 32 query heads, 8 KV heads, head_dim 64.
  q = x @ Wq.T ; k = x @ Wk.T ; v = x @ Wv.T  (nn.Linear convention)
  attn = softmax(q k^T / 8 + causal_mask) v ;  out = attn @ Wo.T
Sharding: tensor-parallel across heads over 8 cores. Core c owns query
heads [4c, 4c+4) and KV head c. Host sums the 8 partial Wo products.

v2 design (all bf16 matmuls, fp32 PSUM):
  - X fed transposed; Q^T = (Wq/8) X^T, fused [K|V]^T = [Wk|Wv] X^T.
  - Q folded via SBUF->SBUF DMA so every head's 64 dims live on
    partitions 0:64 -> K needs no partition duplication.
  - Scores transposed (S^T[k,q] = K Q^T). Causal masking is an additive
    -1e30 bias applied to score PSUM (Pool engine) before exp.
  - PV uses probs as the *stationary* operand and [V|ones] as moving:
    out[q, 0:64] = attn numerator, out[q, 64] = softmax denominator.
    Cost per accumulation step is 65 cols instead of 512, and the
    denominator is per-partition so normalize is a cheap per-partition
    reciprocal + scale (no broadcast matmuls).
  - PV skips key chunks above the causal boundary at 128 granularity.
  - attn [q, d] tiles are PE-transposed back to [d, q] for the Wo
    matmul; output partials are written as bf16 (host sums in fp32).
"""

import numpy as np
import ml_dtypes

import concourse.bass as bass
import concourse.mybir as mybir
from concourse import bacc
from concourse.tile import TileContext
from concourse.bass_utils import run_bass_kernel_spmd

F32 = mybir.dt.float32
F32R = mybir.dt.float32r
BF16 = mybir.dt.bfloat16

H = 2048
S = 4096
NH = 32
NKV = 8
HD = 64
NCORES = 8
QD = 256          # query dims per core (4 heads x 64)
NSC = 8           # 512-wide sequence chunks
SC = 512          # seq chunk width
KC = 128          # key chunk width
HB = H // 128     # 16 contraction blocks

EXP = mybir.ActivationFunctionType.Exp


def _build_nc(reps: int = 1):
    nc = bacc.Bacc("TRN2", target_bir_lowering=False, debug=False,
                   num_devices=NCORES)

    xt = nc.declare_dram_parameter("xt", [H, S], BF16, isOutput=False)
    wq = nc.declare_dram_parameter("wq", [H, QD], BF16, isOutput=False)
    wkv = nc.declare_dram_parameter("wkv", [H, 128], BF16, isOutput=False)
    wo = nc.declare_dram_parameter("wo", [QD, H], BF16, isOutput=False)
    masks = nc.declare_dram_parameter("masks", [128, 4, SC], BF16,
                                      isOutput=False)
    eye = nc.declare_dram_parameter("eye", [128, 192], BF16, isOutput=False)
    out = nc.declare_dram_parameter("out", [S, H], BF16, isOutput=True)

    xt_r = xt.rearrange("(hb p) s -> p hb s", p=128)
    wq_r = wq.rearrange("(hb p) m -> p hb m", p=128)
    wkv_r = wkv.rearrange("(hb p) m -> p hb m", p=128)
    wo_r = wo.rearrange("(b p) n -> p b n", p=128)

    with TileContext(nc) as tc, \
         tc.tile_pool(name="persist", bufs=1) as persist, \
         tc.tile_pool(name="xtp", bufs=3) as xtp, \
         tc.tile_pool(name="qfp", bufs=2) as qfp, \
         tc.tile_pool(name="stgp", bufs=2) as stgp, \
         tc.tile_pool(name="expp", bufs=6) as expp, \
         tc.tile_pool(name="attp", bufs=2) as attp, \
         tc.tile_pool(name="atsp", bufs=2) as atsp, \
         tc.tile_pool(name="recp", bufs=4) as recp, \
         tc.tile_pool(name="outp", bufs=4) as outp, \
         tc.tile_pool(name="ps_sc", bufs=2, space="PSUM") as ps_sc, \
         tc.tile_pool(name="ps_pv", bufs=2, space="PSUM") as ps_pv, \
         tc.tile_pool(name="ps_mix", bufs=2, space="PSUM") as ps_mix:

        # ---- persistent tiles ----
        wq_sb = persist.tile([128, HB, QD], BF16)
        wkv_sb = persist.tile([128, HB, 128], BF16)
        wo_sb = persist.tile([128, 2, H], BF16)
        mask_sb = persist.tile([128, 4, SC], BF16)
        eye_sb = persist.tile([128, 192], BF16)
        kt_sb = persist.tile([64, S], BF16)            # K^T (base partitions)
        vbuf = persist.tile([128, S // KC, 65], BF16)  # [keys, V dims | ones]

        # ---------- emission helpers ----------
        xt_tiles = {}

        def emit_xt_dma(s, split=1):
            q0 = s * SC
            lo = xtp.tile([128, HB // 2, SC], BF16, tag="xt", name="xt_lo")
            hi = xtp.tile([128, HB // 2, SC], BF16, tag="xt", name="xt_hi")
            step = (HB // 2) // split
            for i in range(split):
                h0 = i * step
                nc.sync.dma_start(lo[:, h0:h0 + step, :],
                                  xt_r[:, h0:h0 + step, q0:q0 + SC])
            for i in range(split):
                h0 = i * step
                nc.sync.dma_start(hi[:, h0:h0 + step, :],
                                  xt_r[:, HB // 2 + h0:HB // 2 + h0 + step,
                                       q0:q0 + SC])
            xt_tiles[s] = (lo, hi)

        nc.gpsimd.memset(vbuf[:, :, 64:65], 1.0)

        def emit_startup_dmas():
            """First-needed-first: interleave chunk-0 x pieces with weight
            blocks so the first Q chain starts after ~2 contraction blocks."""
            lo = xtp.tile([128, HB // 2, SC], BF16, tag="xt", name="xt_lo")
            hi = xtp.tile([128, HB // 2, SC], BF16, tag="xt", name="xt_hi")
            nc.sync.dma_start(wq_sb[:, 0:2, :], wq_r[:, 0:2, :])
            for i in range(4):
                nc.sync.dma_start(lo[:, 2 * i:2 * i + 2, :],
                                  xt_r[:, 2 * i:2 * i + 2, 0:SC])
                h0 = 2 + 4 * i
                nc.sync.dma_start(wq_sb[:, h0:min(h0 + 4, HB), :],
                                  wq_r[:, h0:min(h0 + 4, HB), :])
            nc.sync.dma_start(hi[:, 0:4, :], xt_r[:, HB // 2:HB // 2 + 4, 0:SC])
            nc.sync.dma_start(wkv_sb[:], wkv_r)
            nc.sync.dma_start(hi[:, 4:8, :], xt_r[:, HB // 2 + 4:HB, 0:SC])
            nc.sync.dma_start(eye_sb[:], eye[:])
            nc.sync.dma_start(mask_sb[:], masks[:])
            nc.sync.dma_start(wo_sb[:], wo_r)
            xt_tiles[0] = (lo, hi)

        def xchunk(s, hb):
            lo, hi = xt_tiles[s]
            return lo[:, hb, :] if hb < HB // 2 else hi[:, hb - HB // 2, :]

        qf_tiles = {}
        attn_tiles = {}

        def proj_quanta(s):
            """Emission quanta for chunk s's projections."""
            q0 = s * SC
            qf = qfp.tile([64, 4, SC], BF16, tag="qf")
            qf_tiles[s] = qf

            # Q^T: two 128-wide chains; heads 2m at partitions 0:64,
            # heads 2m+1 folded down via SBUF->SBUF DMA.
            for m in range(2):
                ps = ps_mix.tile([128, SC], F32, tag="mix", name="ps_qt")
                for h0 in range(0, HB, 2):
                    for hb in range(h0, h0 + 2):
                        nc.tensor.matmul(ps[:],
                                         wq_sb[:, hb, m * 128:(m + 1) * 128],
                                         xchunk(s, hb),
                                         start=(hb == 0), stop=(hb == HB - 1))
                    yield
                nc.vector.tensor_copy(qf[:, 2 * m, :], ps[0:64, :])
                qs = stgp.tile([128, SC], BF16, tag="qs", name="qstage")
                nc.vector.tensor_copy(qs[64:128, :], ps[64:128, :])
                # fold odd head down to partitions 0:64 on the Pool queue
                # (keeps it off the serial SP queue behind bulk traffic)
                nc.gpsimd.dma_start(qf[:, 2 * m + 1, :], qs[64:128, :])

            # fused [K|V]^T chain: K at partitions 0:64, V at 64:128
            ps2 = ps_mix.tile([128, SC], F32, tag="mix", name="ps_kv")
            for h0 in range(0, HB, 2):
                for hb in range(h0, h0 + 2):
                    nc.tensor.matmul(ps2[:], wkv_sb[:, hb, :], xchunk(s, hb),
                                     start=(hb == 0), stop=(hb == HB - 1))
                yield
            nc.vector.tensor_copy(kt_sb[:, q0:q0 + SC], ps2[0:64, :])
            v16 = stgp.tile([128, SC], BF16, tag="v16", name="v16")
            nc.vector.tensor_copy(v16[64:128, :], ps2[64:128, :])

            # V transpose into vbuf: [64, 128] -> [128, 64] per key chunk
            for t in range(SC // KC):
                kc = (q0 // KC) + t
                pst = ps_mix.tile([128, 64], BF16, tag="mix", name="ps_vtr")
                nc.tensor.transpose(pst[:], v16[64:128, t * KC:(t + 1) * KC],
                                    eye_sb[64:128, 128:192])
                nc.vector.tensor_copy(vbuf[:, kc, 0:64], pst[:])
                if t % 2 == 1:
                    yield

        def wo_quanta(s):
            """Emission quanta for chunk s's output projection."""
            attn_sb = attn_tiles[s]
            q0 = s * SC
            for t in range(SC // 128):
                r0 = q0 + t * 128
                for nck in range(H // SC):
                    ps4 = ps_mix.tile([128, SC], F32, tag="mix", name="ps_wo")
                    for b in range(2):
                        nc.tensor.matmul(
                            ps4[:], attn_sb[:, b, t * 128:(t + 1) * 128],
                            wo_sb[:, b, nck * SC:(nck + 1) * SC],
                            start=(b == 0), stop=(b == 1))
                    o_sb = outp.tile([128, SC], BF16, tag="out")
                    nc.vector.tensor_copy(o_sb[:], ps4[:])
                    # spread output DMA across SP and Pool queues
                    eng = nc.sync if nck % 2 == 0 else nc.gpsimd
                    eng.dma_start(
                        out[r0:r0 + 128, nck * SC:(nck + 1) * SC], o_sb[:])
                    yield

        def attention(s, fillers, n_fill):
            """Attention for query chunk s. Transposed scores in 128-key
            chunk pairs, exp on ACT, 0/1 causal mask multiply on SBUF exp
            tiles (Pool), then PV with probs stationary accumulating
            [q, V|ones] into one PSUM bank per head. All (head, pair)
            items form one flat stream with one-item score/exp lookahead
            so PE never waits for exp at head boundaries."""

            def fill(k):
                while k > 0 and fillers:
                    try:
                        next(fillers[0])
                        k -= 1
                    except StopIteration:
                        fillers.pop(0)

            attn_sb = attp.tile([128, 2, SC], BF16, tag="attn")
            attn_tiles[s] = attn_sb
            qf = qf_tiles[s]
            npairs = 2 * s + 2
            items = [(h, p) for h in range(4) for p in range(npairs)]
            rate = n_fill / float(len(items))
            facc = 0.0

            att_t = {}
            for m in range(2):
                att_t[m] = atsp.tile([128, 4, 2, 64], BF16, tag="att",
                                     name="att_t")

            exps = {}

            def emit_item(i):
                h, p = items[i]
                sc = ps_sc.tile([128, 2, SC], F32, tag="sc", name="sc")
                ex = expp.tile([128, 2, SC], BF16, tag="exp", name="ex")
                diag = p >= npairs - 2
                joff = (p - (npairs - 2)) * 2 if diag else 0
                for j in range(2):
                    kc = 2 * p + j
                    # causally-valid query range of this key chunk
                    q0 = KC * (joff + j) if diag else 0
                    nc.tensor.matmul(sc[:, j, q0:SC],
                                     kt_sb[:, kc * KC:(kc + 1) * KC],
                                     qf[:, h, q0:SC], start=True, stop=True)
                if not diag:
                    nc.scalar.activation(ex[:], sc[:], EXP)
                else:
                    for j in range(2):
                        q0 = KC * (joff + j)
                        nc.scalar.activation(ex[:, j, q0:SC],
                                             sc[:, j, q0:SC], EXP)
                        # 0/1 triangle mask on the partial block
                        nc.gpsimd.tensor_mul(
                            ex[:, j, q0:q0 + KC], ex[:, j, q0:q0 + KC],
                            mask_sb[:, 0, 0:KC])
                exps[i] = ex

            pv_tiles = {}
            ends = {}
            for h in range(4):
                nonskip = []
                for p in range(npairs):
                    for j in range(2):
                        kc = 2 * p + j
                        for qb in range(4):
                            if kc <= 4 * s + qb:
                                nonskip.append((p, j, qb))
                ends[h] = (nonskip[0], nonskip[-1])

            emit_item(0)
            for i, (h, p) in enumerate(items):
                m, hp = h // 2, h % 2
                if i + 1 < len(items):
                    emit_item(i + 1)
                facc += rate
                nf = int(facc)
                facc -= nf
                fill(nf)
                if p == 0:
                    pv_tiles[h] = ps_pv.tile([128, 4, 65], F32, tag="pv",
                                             name="pv")
                pv = pv_tiles[h]
                first_pjq, last_pjq = ends[h]
                ex = exps.pop(i)
                for j in range(2):
                    kc = 2 * p + j
                    for qb in range(4):
                        if kc > 4 * s + qb:
                            continue
                        nc.tensor.matmul(
                            pv[:, qb, :],
                            ex[:, j, qb * 128:(qb + 1) * 128],
                            vbuf[:, kc, :],
                            start=((p, j, qb) == first_pjq),
                            stop=((p, j, qb) == last_pjq))

                if p == npairs - 1:
                    # normalize: per-partition reciprocal of the ones col
                    rec = recp.tile([128, 4], F32, tag="rec", name="rec")
                    with nc.allow_low_precision(reason="softmax recip"):
                        nc.vector.reciprocal(rec[:], pv[:, :, 64])
                    for qb in range(4):
                        nc.vector.tensor_scalar_mul(att_t[m][:, qb, hp, :],
                                                    pv[:, qb, 0:64],
                                                    rec[:, qb:qb + 1])
                    if hp == 1:
                        # transpose [q, 2 heads x 64] -> [128 dims, q]
                        for qb in range(4):
                            pst = ps_mix.tile([128, 128], BF16, tag="mix",
                                              name="ps_atr")
                            nc.tensor.transpose(pst[:],
                                                att_t[m][:, qb, :, :],
                                                eye_sb[:, 0:128])
                            nc.vector.tensor_copy(
                                attn_sb[:, m, qb * 128:(qb + 1) * 128],
                                pst[:])

        def body(first=True):
            if first:
                emit_startup_dmas()
            else:
                emit_xt_dma(0)
            for _ in proj_quanta(0):
                pass
            for s in range(NSC):
                fillers = []
                n_fill = 0
                if s + 1 < NSC:
                    emit_xt_dma(s + 1)
                    fillers.append(proj_quanta(s + 1))
                    n_fill += 26
                if s - 1 >= 0:
                    fillers.append(wo_quanta(s - 1))
                    n_fill += 16
                attention(s, fillers, n_fill)
                for f in list(fillers):
                    for _ in f:
                        pass
            for _ in wo_quanta(NSC - 1):
                pass

        # manual unroll for timing runs (tc.For_i chokes the tile
        # scheduler on this kernel's cross-iteration tile reuse)
        for r in range(reps):
            body(first=(r == 0))

    nc.compile()
    return nc


_NC_CACHE = None


def _get_nc():
    global _NC_CACHE
    if _NC_CACHE is None:
        _NC_CACHE = _build_nc()
    return _NC_CACHE


def _make_in_maps(hidden_states, Wq, Wk, Wv, Wo):
    bf = ml_dtypes.bfloat16
    x = np.asarray(hidden_states, dtype=np.float32).reshape(S, H)
    xt = np.ascontiguousarray(x.T).astype(bf)

    # 0/1 causal masks for diagonal 128-key chunks:
    # mask[k, j, q] = 0 where q < 128*j + k else 1
    k_idx = np.arange(128)[:, None]
    q_idx = np.arange(SC)[None, :]
    masks = np.stack(
        [np.where(q_idx < 128 * j + k_idx, 0.0, 1.0) for j in range(4)],
        axis=1)
    masks = np.ascontiguousarray(masks).astype(bf)

    eye = np.zeros((128, 192), np.float32)
    eye[:, 0:128] = np.eye(128)
    eye[64:128, 128:192] = np.eye(64)
    eye = eye.astype(bf)

    in_maps = []
    for c in range(NCORES):
        wq_c = np.ascontiguousarray(
            (Wq[QD * c:QD * (c + 1), :] / 8.0).T).astype(bf)
        wkv_c = np.concatenate([Wk[HD * c:HD * (c + 1), :],
                                Wv[HD * c:HD * (c + 1), :]], axis=0)
        wkv_c = np.ascontiguousarray(wkv_c.T).astype(bf)
        wo_c = np.ascontiguousarray(Wo[:, QD * c:QD * (c + 1)].T).astype(bf)
        in_maps.append({
            "xt": xt, "wq": wq_c, "wkv": wkv_c, "wo": wo_c,
            "masks": masks, "eye": eye,
        })
    return in_maps


def kernel(hidden_states, attention_mask, Wq, Wk, Wv, Wo, _trace=False):
    nc = _get_nc()
    in_maps = _make_in_maps(hidden_states, Wq, Wk, Wv, Wo)
    res = run_bass_kernel_spmd(nc, in_maps, core_ids=list(range(NCORES)),
                               trace=_trace)
    kernel.last_results = res
    total = res.results[0]["out"].astype(np.float32)
    for c in range(1, NCORES):
        total += res.results[c]["out"].astype(np.float32)
    return total.reshape(1, S, H)


# revision 23
# speedup vs baseline: 51410.4307x; 1.0194x over previous
"""Trainium2 Bass kernel for ConvaiCausalLMAttention (GQA causal attention).

Problem: B=1, S=4096, H=2048, 32 query heads, 8 KV heads, head_dim 64.
  q = x @ Wq.T ; k = x @ Wk.T ; v = x @ Wv.T  (nn.Linear convention)
  attn = softmax(q k^T / 8 + causal_mask) v ;  out = attn @ Wo.T
Sharding: tensor-parallel across heads over 8 cores. Core c owns query
heads [4c, 4c+4) and KV head c. Host sums the 8 partial Wo products.

v2 design (all bf16 matmuls, fp32 PSUM):
  - X fed transposed; Q^T = (Wq/8) X^T, fused [K|V]^T = [Wk|Wv] X^T.
  - Q folded via SBUF->SBUF DMA so every head's 64 dims live on
    partitions 0:64 -> K needs no partition duplication.
  - Scores transposed (S^T[k,q] = K Q^T). Causal masking is an additive
    -1e30 bias applied to score PSUM (Pool engine) before exp.
  - PV uses probs as the *stationary* operand and [V|ones] as moving:
    out[q, 0:64] = attn numerator, out[q, 64] = softmax denominator.
    Cost per accumulation step is 65 cols instead of 512, and the
    denominator is per-partition so normalize is a cheap per-partition
    reciprocal + scale (no broadcast matmuls).
  - PV skips key chunks above the causal boundary at 128 granularity.
  - attn [q, d] tiles are PE-transposed back to [d, q] for the Wo
    matmul; output partials are written as bf16 (host sums in fp32).
"""

import numpy as np
import ml_dtypes

import concourse.bass as bass
import concourse.mybir as mybir
from concourse import bacc
from concourse.tile import TileContext
from concourse.bass_utils import run_bass_kernel_spmd

F32 = mybir.dt.float32
F32R = mybir.dt.float32r
BF16 = mybir.dt.bfloat16

H = 2048
S = 4096
NH = 32
NKV = 8
HD = 64
NCORES = 8
QD = 256          # query dims per core (4 heads x 64)
NSC = 8           # 512-wide sequence chunks
SC = 512          # seq chunk width
KC = 128          # key chunk width
HB = H // 128     # 16 contraction blocks

EXP = mybir.ActivationFunctionType.Exp


def _build_nc(reps: int = 1):
    nc = bacc.Bacc("TRN2", target_bir_lowering=False, debug=False,
                   num_devices=NCORES)

    xt = nc.declare_dram_parameter("xt", [H, S], BF16, isOutput=False)
    wq = nc.declare_dram_parameter("wq", [H, QD], BF16, isOutput=False)
    wkv = nc.declare_dram_parameter("wkv", [H, 128], BF16, isOutput=False)
    wo = nc.declare_dram_parameter("wo", [QD, H], BF16, isOutput=False)
    masks = nc.declare_dram_parameter("masks", [128, 4, SC], BF16,
                                      isOutput=False)
    eye = nc.declare_dram_parameter("eye", [128, 192], BF16, isOutput=False)
    out = nc.declare_dram_parameter("out", [S, H], BF16, isOutput=True)

    xt_r = xt.rearrange("(hb p) s -> p hb s", p=128)
    wq_r = wq.rearrange("(hb p) m -> p hb m", p=128)
    wkv_r = wkv.rearrange("(hb p) m -> p hb m", p=128)
    wo_r = wo.rearrange("(b p) n -> p b n", p=128)

    with TileContext(nc) as tc, \
         tc.tile_pool(name="persist", bufs=1) as persist, \
         tc.tile_pool(name="xtp", bufs=3) as xtp, \
         tc.tile_pool(name="qfp", bufs=2) as qfp, \
         tc.tile_pool(name="stgp", bufs=2) as stgp, \
         tc.tile_pool(name="expp", bufs=6) as expp, \
         tc.tile_pool(name="attp", bufs=2) as attp, \
         tc.tile_pool(name="atsp", bufs=2) as atsp, \
         tc.tile_pool(name="recp", bufs=4) as recp, \
         tc.tile_pool(name="outp", bufs=4) as outp, \
         tc.tile_pool(name="ps_sc", bufs=2, space="PSUM") as ps_sc, \
         tc.tile_pool(name="ps_pv", bufs=2, space="PSUM") as ps_pv, \
         tc.tile_pool(name="ps_mix", bufs=2, space="PSUM") as ps_mix:

        # ---- persistent tiles ----
        wq_sb = persist.tile([128, HB, QD], BF16)
        wkv_sb = persist.tile([128, HB, 128], BF16)
        wo_sb = persist.tile([128, 2, H], BF16)
        mask_sb = persist.tile([128, 4, SC], BF16)
        eye_sb = persist.tile([128, 192], BF16)
        kt_sb = persist.tile([64, S], BF16)            # K^T (base partitions)
        vbuf = persist.tile([128, S // KC, 65], BF16)  # [keys, V dims | ones]

        # ---------- emission helpers ----------
        xt_tiles = {}

        def emit_xt_dma(s, split=1):
            q0 = s * SC
            lo = xtp.tile([128, HB // 2, SC], BF16, tag="xt", name="xt_lo")
            hi = xtp.tile([128, HB // 2, SC], BF16, tag="xt", name="xt_hi")
            step = (HB // 2) // split
            for i in range(split):
                h0 = i * step
                nc.sync.dma_start(lo[:, h0:h0 + step, :],
                                  xt_r[:, h0:h0 + step, q0:q0 + SC])
            # hi half rides the Pool queue in parallel
            for i in range(split):
                h0 = i * step
                nc.gpsimd.dma_start(hi[:, h0:h0 + step, :],
                                    xt_r[:, HB // 2 + h0:HB // 2 + h0 + step,
                                         q0:q0 + SC])
            xt_tiles[s] = (lo, hi)

        nc.gpsimd.memset(vbuf[:, :, 64:65], 1.0)

        def emit_startup_dmas():
            """First-needed-first: interleave chunk-0 x pieces with weight
            blocks so the first Q chain starts after ~2 contraction blocks."""
            lo = xtp.tile([128, HB // 2, SC], BF16, tag="xt", name="xt_lo")
            hi = xtp.tile([128, HB // 2, SC], BF16, tag="xt", name="xt_hi")
            # supply matches the Q chain's hb consumption order exactly:
            # wq block for hb pair, then the x piece it multiplies
            nc.sync.dma_start(wq_sb[:, 0:2, :], wq_r[:, 0:2, :])
            for i in range(4):
                nc.sync.dma_start(lo[:, 2 * i:2 * i + 2, :],
                                  xt_r[:, 2 * i:2 * i + 2, 0:SC])
                nc.sync.dma_start(wq_sb[:, 2 * i + 2:2 * i + 4, :],
                                  wq_r[:, 2 * i + 2:2 * i + 4, :])
            # second DMA queue (Pool) carries the hi half + K/V weights in
            # parallel with the SP queue's lo half + Q weights
            for i in range(4):
                nc.gpsimd.dma_start(
                    hi[:, 2 * i:2 * i + 2, :],
                    xt_r[:, HB // 2 + 2 * i:HB // 2 + 2 * i + 2, 0:SC])
            for i in range(3):
                h0 = 10 + 2 * i
                nc.sync.dma_start(wq_sb[:, h0:h0 + 2, :],
                                  wq_r[:, h0:h0 + 2, :])
            nc.gpsimd.dma_start(wkv_sb[:], wkv_r)
            nc.sync.dma_start(eye_sb[:], eye[:])
            nc.sync.dma_start(mask_sb[:], masks[:])
            nc.gpsimd.dma_start(wo_sb[:], wo_r)
            xt_tiles[0] = (lo, hi)

        def xchunk(s, hb):
            lo, hi = xt_tiles[s]
            return lo[:, hb, :] if hb < HB // 2 else hi[:, hb - HB // 2, :]

        qf_tiles = {}
        attn_tiles = {}

        def proj_quanta(s):
            """Emission quanta for chunk s's projections."""
            q0 = s * SC
            qf = qfp.tile([64, 4, SC], BF16, tag="qf")
            qf_tiles[s] = qf

            # Q^T: two 128-wide chains; heads 2m at partitions 0:64,
            # heads 2m+1 folded down via SBUF->SBUF DMA.
            for m in range(2):
                ps = ps_mix.tile([128, SC], F32, tag="mix", name="ps_qt")
                for h0 in range(0, HB, 2):
                    for hb in range(h0, h0 + 2):
                        nc.tensor.matmul(ps[:],
                                         wq_sb[:, hb, m * 128:(m + 1) * 128],
                                         xchunk(s, hb),
                                         start=(hb == 0), stop=(hb == HB - 1))
                    yield
                nc.vector.tensor_copy(qf[:, 2 * m, :], ps[0:64, :])
                qs = stgp.tile([128, SC], BF16, tag="qs", name="qstage")
                nc.vector.tensor_copy(qs[64:128, :], ps[64:128, :])
                # fold odd head down to partitions 0:64 on the Pool queue
                # (keeps it off the serial SP queue behind bulk traffic)
                nc.gpsimd.dma_start(qf[:, 2 * m + 1, :], qs[64:128, :])

            # fused [K|V]^T chain: K at partitions 0:64, V at 64:128
            ps2 = ps_mix.tile([128, SC], F32, tag="mix", name="ps_kv")
            for h0 in range(0, HB, 2):
                for hb in range(h0, h0 + 2):
                    nc.tensor.matmul(ps2[:], wkv_sb[:, hb, :], xchunk(s, hb),
                                     start=(hb == 0), stop=(hb == HB - 1))
                yield
            nc.vector.tensor_copy(kt_sb[:, q0:q0 + SC], ps2[0:64, :])
            v16 = stgp.tile([128, SC], BF16, tag="v16", name="v16")
            nc.vector.tensor_copy(v16[64:128, :], ps2[64:128, :])

            # V transpose into vbuf: [64, 128] -> [128, 64] per key chunk
            for t in range(SC // KC):
                kc = (q0 // KC) + t
                pst = ps_mix.tile([128, 64], BF16, tag="mix", name="ps_vtr")
                nc.tensor.transpose(pst[:], v16[64:128, t * KC:(t + 1) * KC],
                                    eye_sb[64:128, 128:192])
                nc.vector.tensor_copy(vbuf[:, kc, 0:64], pst[:])
                if t % 2 == 1:
                    yield

        def wo_quanta(s):
            """Emission quanta for chunk s's output projection."""
            attn_sb = attn_tiles[s]
            q0 = s * SC
            for t in range(SC // 128):
                r0 = q0 + t * 128
                for nck in range(H // SC):
                    ps4 = ps_mix.tile([128, SC], F32, tag="mix", name="ps_wo")
                    for b in range(2):
                        nc.tensor.matmul(
                            ps4[:], attn_sb[:, b, t * 128:(t + 1) * 128],
                            wo_sb[:, b, nck * SC:(nck + 1) * SC],
                            start=(b == 0), stop=(b == 1))
                    o_sb = outp.tile([128, SC], BF16, tag="out")
                    nc.vector.tensor_copy(o_sb[:], ps4[:])
                    # spread output DMA across queues; the last chunk also
                    # uses the (by then idle) ACT queue to shrink the drain
                    if s == NSC - 1:
                        eng = (nc.sync, nc.gpsimd, nc.scalar)[nck % 3]
                    else:
                        eng = nc.sync if nck % 2 == 0 else nc.gpsimd
                    eng.dma_start(
                        out[r0:r0 + 128, nck * SC:(nck + 1) * SC], o_sb[:])
                    yield

        def attention(s, fillers, n_fill):
            """Attention for query chunk s. Transposed scores in 128-key
            chunk pairs, exp on ACT, 0/1 causal mask multiply on SBUF exp
            tiles (Pool), then PV with probs stationary accumulating
            [q, V|ones] into one PSUM bank per head. All (head, pair)
            items form one flat stream with one-item score/exp lookahead
            so PE never waits for exp at head boundaries."""

            def fill(k):
                while k > 0 and fillers:
                    try:
                        next(fillers[0])
                        k -= 1
                    except StopIteration:
                        fillers.pop(0)

            attn_sb = attp.tile([128, 2, SC], BF16, tag="attn")
            attn_tiles[s] = attn_sb
            qf = qf_tiles[s]
            npairs = 2 * s + 2
            items = [(h, p) for h in range(4) for p in range(npairs)]
            rate = n_fill / float(len(items))
            facc = 0.0

            att_t = {}
            for m in range(2):
                att_t[m] = atsp.tile([128, 4, 2, 64], BF16, tag="att",
                                     name="att_t")

            exps = {}

            def emit_item(i):
                h, p = items[i]
                sc = ps_sc.tile([128, 2, SC], F32, tag="sc", name="sc")
                ex = expp.tile([128, 2, SC], BF16, tag="exp", name="ex")
                diag = p >= npairs - 2
                joff = (p - (npairs - 2)) * 2 if diag else 0
                for j in range(2):
                    kc = 2 * p + j
                    # causally-valid query range of this key chunk
                    q0 = KC * (joff + j) if diag else 0
                    nc.tensor.matmul(sc[:, j, q0:SC],
                                     kt_sb[:, kc * KC:(kc + 1) * KC],
                                     qf[:, h, q0:SC], start=True, stop=True)
                if not diag:
                    nc.scalar.activation(ex[:], sc[:], EXP)
                else:
                    for j in range(2):
                        q0 = KC * (joff + j)
                        nc.scalar.activation(ex[:, j, q0:SC],
                                             sc[:, j, q0:SC], EXP)
                        # 0/1 triangle mask on the partial block
                        nc.gpsimd.tensor_mul(
                            ex[:, j, q0:q0 + KC], ex[:, j, q0:q0 + KC],
                            mask_sb[:, 0, 0:KC])
                exps[i] = ex

            pv_tiles = {}
            ends = {}
            for h in range(4):
                nonskip = []
                for p in range(npairs):
                    for j in range(2):
                        kc = 2 * p + j
                        for qb in range(4):
                            if kc <= 4 * s + qb:
                                nonskip.append((p, j, qb))
                ends[h] = (nonskip[0], nonskip[-1])

            emit_item(0)
            for i, (h, p) in enumerate(items):
                m, hp = h // 2, h % 2
                if i + 1 < len(items):
                    emit_item(i + 1)
                facc += rate
                nf = int(facc)
                facc -= nf
                fill(nf)
                if p == 0:
                    pv_tiles[h] = ps_pv.tile([128, 4, 65], F32, tag="pv",
                                             name="pv")
                pv = pv_tiles[h]
                first_pjq, last_pjq = ends[h]
                ex = exps.pop(i)
                for j in range(2):
                    kc = 2 * p + j
                    for qb in range(4):
                        if kc > 4 * s + qb:
                            continue
                        nc.tensor.matmul(
                            pv[:, qb, :],
                            ex[:, j, qb * 128:(qb + 1) * 128],
                            vbuf[:, kc, :],
                            start=((p, j, qb) == first_pjq),
                            stop=((p, j, qb) == last_pjq))

                if p == npairs - 1:
                    # normalize: per-partition reciprocal of the ones col
                    rec = recp.tile([128, 4], F32, tag="rec", name="rec")
                    with nc.allow_low_precision(reason="softmax recip"):
                        nc.vector.reciprocal(rec[:], pv[:, :, 64])
                    for qb in range(4):
                        nc.vector.tensor_scalar_mul(att_t[m][:, qb, hp, :],
                                                    pv[:, qb, 0:64],
                                                    rec[:, qb:qb + 1])
                    if hp == 1:
                        # transpose [q, 2 heads x 64] -> [128 dims, q]
                        for qb in range(4):
                            pst = ps_mix.tile([128, 128], BF16, tag="mix",
                                              name="ps_atr")
                            nc.tensor.transpose(pst[:],
                                                att_t[m][:, qb, :, :],
                                                eye_sb[:, 0:128])
                            nc.vector.tensor_copy(
                                attn_sb[:, m, qb * 128:(qb + 1) * 128],
                                pst[:])

        def body(first=True):
            if first:
                emit_startup_dmas()
            else:
                emit_xt_dma(0)
            for _ in proj_quanta(0):
                pass
            for s in range(NSC):
                fillers = []
                n_fill = 0
                if s + 1 < NSC:
                    emit_xt_dma(s + 1)
                    fillers.append(proj_quanta(s + 1))
                    n_fill += 26
                if s - 1 >= 0:
                    fillers.append(wo_quanta(s - 1))
                    n_fill += 16
                attention(s, fillers, n_fill)
                for f in list(fillers):
                    for _ in f:
                        pass
            for _ in wo_quanta(NSC - 1):
                pass

        # manual unroll for timing runs (tc.For_i chokes the tile
        # scheduler on this kernel's cross-iteration tile reuse)
        for r in range(reps):
            body(first=(r == 0))

    nc.compile()
    return nc


_NC_CACHE = None


def _get_nc():
    global _NC_CACHE
    if _NC_CACHE is None:
        _NC_CACHE = _build_nc()
    return _NC_CACHE


def _make_in_maps(hidden_states, Wq, Wk, Wv, Wo):
    bf = ml_dtypes.bfloat16
    x = np.asarray(hidden_states, dtype=np.float32).reshape(S, H)
    xt = np.ascontiguousarray(x.T).astype(bf)

    # 0/1 causal masks for diagonal 128-key chunks:
    # mask[k, j, q] = 0 where q < 128*j + k else 1
    k_idx = np.arange(128)[:, None]
    q_idx = np.arange(SC)[None, :]
    masks = np.stack(
        [np.where(q_idx < 128 * j + k_idx, 0.0, 1.0) for j in range(4)],
        axis=1)
    masks = np.ascontiguousarray(masks).astype(bf)

    eye = np.zeros((128, 192), np.float32)
    eye[:, 0:128] = np.eye(128)
    eye[64:128, 128:192] = np.eye(64)
    eye = eye.astype(bf)

    in_maps = []
    for c in range(NCORES):
        wq_c = np.ascontiguousarray(
            (Wq[QD * c:QD * (c + 1), :] / 8.0).T).astype(bf)
        wkv_c = np.concatenate([Wk[HD * c:HD * (c + 1), :],
                                Wv[HD * c:HD * (c + 1), :]], axis=0)
        wkv_c = np.ascontiguousarray(wkv_c.T).astype(bf)
        wo_c = np.ascontiguousarray(Wo[:, QD * c:QD * (c + 1)].T).astype(bf)
        in_maps.append({
            "xt": xt, "wq": wq_c, "wkv": wkv_c, "wo": wo_c,
            "masks": masks, "eye": eye,
        })
    return in_maps


def kernel(hidden_states, attention_mask, Wq, Wk, Wv, Wo, _trace=False):
    nc = _get_nc()
    in_maps = _make_in_maps(hidden_states, Wq, Wk, Wv, Wo)
    res = run_bass_kernel_spmd(nc, in_maps, core_ids=list(range(NCORES)),
                               trace=_trace)
    kernel.last_results = res
    total = res.results[0]["out"].astype(np.float32)
    for c in range(1, NCORES):
        total += res.results[c]["out"].astype(np.float32)
    return total.reshape(1, S, H)
